# revision 2
# baseline (speedup 1.0000x reference)
"""MQA attention kernel for Trainium2, sharded over 8 NeuronCores.

Problem: query [1, 2048, 16, 128] f32, shared key/value [1, 2048, 128] f32,
mask [1, 16, 2048, 2048] bool (all ones -> no-op, per problem spec fill).

Sharding: tensor-parallel over heads, 2 heads per core; K/V replicated.

Per-core kernel (per head):
  - scores^T tiles: S^T[kv, q] = K^T(stationary) @ Q^T(moving), fp32r matmuls
    (kv tiled by 128, q by 512), K=d=128 contraction on partitions.
  - P^T = exp(SCALE * S^T) on ScalarE, fp32 PSUM -> fp16 SBUF.
  - PV: out[q, 0:128] = attention numerator, out[q, 128] = softmax denominator,
    computed in ONE accumulation group per q-chunk with lhsT = P^T tile
    (stationary) and rhs = [V | ones] (moving, fp16). No on-chip transposes.
  - normalize with DVE reciprocal + tensor_scalar_mul while evacuating PSUM.

Host side: pre-transposes Q/K (free on CPU), appends the ones column to V,
scatters per-core inputs, gathers per-core outputs.
"""

import numpy as np

import concourse.bass as bass
import concourse.tile as tile
from concourse import bacc, mybir
from concourse.bass_utils import run_bass_kernel_spmd

N_CORES = 8
H = 16
HPC = H // N_CORES  # heads per core
Q = 2048
KV = 2048
D = 128
P = 128
NKV = KV // P       # 16 kv tiles
NQS = Q // 512      # 4 q chunks for the scores matmul (N=512)
NQO = Q // P        # 16 q chunks for the PV matmul
VA = D + 1          # V augmented with a ones column
SCALE = float(1.0 / np.sqrt(np.float32(D)))

F32 = mybir.dt.float32
F32R = mybir.dt.float32r
F16 = mybir.dt.float16

_CACHE = {}


def _build():
    nc = bacc.Bacc("TRN2", target_bir_lowering=False, debug=False,
                   num_devices=N_CORES)
    qT = nc.dram_tensor("qT", [HPC, P, Q], F32R, kind="ExternalInput")
    kT = nc.dram_tensor("kT", [P, KV], F32R, kind="ExternalInput")
    vaug = nc.dram_tensor("vaug", [P, NKV * VA], F16, kind="ExternalInput")
    o = nc.dram_tensor("o", [HPC, NQO, P, D], F32, kind="ExternalOutput")

    with tile.TileContext(nc) as tc:
        with (
            tc.tile_pool(name="const", bufs=1) as const_pool,
            tc.tile_pool(name="qT", bufs=2) as qT_pool,
            tc.tile_pool(name="pT", bufs=24) as pT_pool,
            tc.tile_pool(name="osb", bufs=2) as osb_pool,
            tc.tile_pool(name="recip", bufs=8) as recip_pool,
            tc.tile_pool(name="psumS", bufs=1, space="PSUM") as psumS_pool,
            tc.tile_pool(name="psumO", bufs=4, space="PSUM") as psumO_pool,
        ):
            kT_sb = const_pool.tile([P, KV], F32R)
            nc.sync.dma_start(kT_sb[:], kT.ap())
            vaug_sb = const_pool.tile([P, NKV * VA], F16)
            nc.sync.dma_start(vaug_sb[:], vaug.ap())

            for h in range(HPC):
                qT_sb = qT_pool.tile([P, Q], F32R)
                nc.sync.dma_start(qT_sb[:], qT.ap()[h])

                # scores^T + exp, one [128, Q] stripe per kv tile
                pTs = []
                for i in range(NKV):
                    ps = psumS_pool.tile([P, Q], F32)
                    for j in range(NQS):
                        nc.tensor.matmul(
                            ps[:, j * 512:(j + 1) * 512],
                            kT_sb[:, i * P:(i + 1) * P],
                            qT_sb[:, j * 512:(j + 1) * 512],
                            start=True, stop=True,
                        )
                    pT = pT_pool.tile([P, Q], F16, tag="pT")
                    nc.scalar.activation(
                        pT[:], ps[:], mybir.ActivationFunctionType.Exp,
                        scale=SCALE,
                    )
                    pTs.append(pT)

                # PV + denominator in one accumulation group per q chunk
                osb = osb_pool.tile([P, Q], F32)
                for j in range(NQO):
                    po = psumO_pool.tile([P, VA], F32)
                    for i in range(NKV):
                        nc.tensor.matmul(
                            po[:],
                            pTs[i][:, j * P:(j + 1) * P],
                            vaug_sb[:, i * VA:(i + 1) * VA],
                            start=(i == 0), stop=(i == NKV - 1),
                        )
                    rc = recip_pool.tile([P, 1], F32)
                    nc.vector.reciprocal(rc[:], po[:, D:D + 1])
                    nc.vector.tensor_scalar_mul(
                        osb[:, j * P:(j + 1) * P], po[:, 0:D], rc[:],
                    )
                nc.sync.dma_start(
                    o.ap()[h].rearrange("j p d -> p j d"),
                    osb[:].rearrange("p (j d) -> p j d", d=D),
                )
    nc.compile()
    return nc


def _get_nc():
    if "nc" not in _CACHE:
        _CACHE["nc"] = _build()
    return _CACHE["nc"]


def kernel(query_states, key_states, value_states, attention_mask):
    # mask is all-ones by problem construction -> identity; ignored.
    q = np.asarray(query_states, dtype=np.float32).reshape(Q, H, D)
    k = np.asarray(key_states, dtype=np.float32).reshape(KV, D)
    v = np.asarray(value_states, dtype=np.float32).reshape(KV, D)

    kT = np.ascontiguousarray(k.T)  # [128, KV]
    # [V | ones] in fp16, laid out [128 kv-local, NKV * 129]
    va = np.concatenate(
        [v.reshape(NKV, P, D), np.ones((NKV, P, 1), np.float32)], axis=2
    ).astype(np.float16)
    vaug = np.ascontiguousarray(va.transpose(1, 0, 2)).reshape(P, NKV * VA)

    in_maps = []
    for c in range(N_CORES):
        qc = q[:, c * HPC:(c + 1) * HPC, :]            # [Q, HPC, D]
        qTc = np.ascontiguousarray(qc.transpose(1, 2, 0))  # [HPC, 128, Q]
        in_maps.append({"qT": qTc, "kT": kT, "vaug": vaug})

    nc = _get_nc()
    res = run_bass_kernel_spmd(nc, in_maps, core_ids=list(range(N_CORES)))

    out = np.empty((Q, H, D), dtype=np.float32)
    for c in range(N_CORES):
        oc = res.results[c]["o"]  # [HPC, NQO, 128, D]
        for hh in range(HPC):
            out[:, c * HPC + hh, :] = oc[hh].reshape(Q, D)
    return out.reshape(1, Q, H, D)


# revision 4
# speedup vs baseline: 1.8153x; 1.8153x over previous
"""MQA attention kernel for Trainium2, sharded over 8 NeuronCores.

Problem: query [1, 2048, 16, 128] f32, shared key/value [1, 2048, 128] f32,
mask [1, 16, 2048, 2048] bool (all ones -> no-op, per problem spec fill).

Sharding: tensor-parallel over heads, 2 heads per core; K/V replicated.

Per-core kernel, software-pipelined over 4 units (head x q-half, q=1024):
  - scores^T stripes: S^T[kv_tile, q_unit] = K^T(stationary) @ Q^T(moving),
    fp32r matmuls, contraction d=128 on partitions.
  - P^T = exp(SCALE * S^T) on ScalarE, fp32 PSUM -> fp16 SBUF.
  - PV: out[q, 0:128] = attention numerator, out[q, 128] = softmax denominator,
    in ONE accumulation group per q-chunk of 128: lhsT = P^T tile (stationary),
    rhs = [V | ones] (moving, fp16). No on-chip transposes anywhere.
  - normalize with DVE reciprocal + tensor_scalar_mul while evacuating PSUM.
Unit u's PV groups are interleaved (in program order) with unit u+1's
scores/exp so the PE stays dense while ScalarE (the exp floor) streams.

Host side: pre-transposes Q/K (free on CPU), appends the ones column to V,
scatters per-core inputs, gathers per-core outputs.
"""

import numpy as np

import concourse.bass as bass
import concourse.tile as tile
from concourse import bacc, mybir
from concourse.bass_utils import run_bass_kernel_spmd

N_CORES = 8
H = 16
HPC = H // N_CORES   # heads per core
Q = 2048
KV = 2048
D = 128
P = 128
QU = 1024            # q extent of one pipeline unit
NU = HPC * Q // QU   # 4 pipeline units per core
NKV = KV // P        # 16 kv tiles
NQS = QU // 512      # 2 scores matmuls (N=512) per psum stripe
NQO = QU // P        # 8 PV q-chunks per unit
VA = D + 1           # V augmented with a ones column
SCALE = float(1.0 / np.sqrt(np.float32(D)))

F32 = mybir.dt.float32
F32R = mybir.dt.float32r
F16 = mybir.dt.float16

_CACHE = {}


def _build():
    nc = bacc.Bacc("TRN2", target_bir_lowering=False, debug=False,
                   num_devices=N_CORES)
    qT = nc.dram_tensor("qT", [NU, P, QU], F32R, kind="ExternalInput")
    kT = nc.dram_tensor("kT", [P, KV], F32R, kind="ExternalInput")
    vaug = nc.dram_tensor("vaug", [P, NKV * VA], F16, kind="ExternalInput")
    o = nc.dram_tensor("o", [NU, NQO, P, D], F32, kind="ExternalOutput")

    with tile.TileContext(nc) as tc:
        with (
            tc.tile_pool(name="const", bufs=1) as const_pool,
            tc.tile_pool(name="qTp", bufs=2) as qT_pool,
            tc.tile_pool(name="pT", bufs=2 * NKV) as pT_pool,
            tc.tile_pool(name="osb", bufs=2) as osb_pool,
            tc.tile_pool(name="recip", bufs=4) as recip_pool,
            tc.tile_pool(name="psumS", bufs=3, space="PSUM") as psumS_pool,
            tc.tile_pool(name="psumO", bufs=2, space="PSUM") as psumO_pool,
        ):
            # split big input DMAs so the first matmul can start early
            kT_sb = const_pool.tile([P, KV], F32R)
            nc.sync.dma_start(kT_sb[:, 0:KV // 2], kT.ap()[:, 0:KV // 2])
            nc.sync.dma_start(kT_sb[:, KV // 2:], kT.ap()[:, KV // 2:])
            vaug_sb = const_pool.tile([P, NKV * VA], F16)
            nc.sync.dma_start(vaug_sb[:], vaug.ap())

            qT_sbs = {}

            def load_q(u):
                t = qT_pool.tile([P, QU], F32R, name="qT_sb", tag="qT")
                nc.sync.dma_start(t[:, 0:QU // 2], qT.ap()[u][:, 0:QU // 2])
                nc.sync.dma_start(t[:, QU // 2:], qT.ap()[u][:, QU // 2:])
                qT_sbs[u] = t

            load_q(0)
            load_q(1)

            pTs = {u: [] for u in range(NU)}
            osbs = {}

            def pv_group(u, j):
                # one PSUM accumulation group: O[q_j, :] plus denominator
                po = psumO_pool.tile([P, VA], F32, name="po", tag="po")
                for i in range(NKV):
                    nc.tensor.matmul(
                        po[:],
                        pTs[u][i][:, j * P:(j + 1) * P],
                        vaug_sb[:, i * VA:(i + 1) * VA],
                        start=(i == 0), stop=(i == NKV - 1),
                    )
                rc = recip_pool.tile([P, 1], F32, name="rc", tag="rc")
                nc.vector.reciprocal(rc[:], po[:, D:D + 1])
                nc.vector.tensor_scalar_mul(
                    osbs[u][:, j * P:(j + 1) * P], po[:, 0:D], rc[:],
                )

            for u in range(NU + 1):
                if u < NU:
                    osbs[u] = osb_pool.tile([P, QU], F32, name="osb", tag="osb")
                for i in range(NKV):
                    # scores + exp for unit u
                    if u < NU:
                        ps = psumS_pool.tile([P, QU], F32, name="ps", tag="ps")
                        for j in range(NQS):
                            nc.tensor.matmul(
                                ps[:, j * 512:(j + 1) * 512],
                                kT_sb[:, i * P:(i + 1) * P],
                                qT_sbs[u][:, j * 512:(j + 1) * 512],
                                start=True, stop=True,
                            )
                        pT = pT_pool.tile([P, QU], F16, name="pT", tag="pT")
                        nc.scalar.activation(
                            pT[:], ps[:], mybir.ActivationFunctionType.Exp,
                            scale=SCALE,
                        )
                        pTs[u].append(pT)
                    # PV for unit u-1, one group every other kv-tile
                    if u > 0 and i % 2 == 0:
                        pv_group(u - 1, i // 2)
                if u == 0:
                    load_q(2)
                if u == 1:
                    load_q(3)
                if u > 0:
                    nc.sync.dma_start(
                        o.ap()[u - 1].rearrange("j p d -> p j d"),
                        osbs[u - 1][:].rearrange("p (j d) -> p j d", d=D),
                    )
                    pTs[u - 1] = []
    nc.compile()
    return nc


def _get_nc():
    if "nc" not in _CACHE:
        _CACHE["nc"] = _build()
    return _CACHE["nc"]


def kernel(query_states, key_states, value_states, attention_mask):
    # mask is all-ones by problem construction -> identity; ignored.
    q = np.asarray(query_states, dtype=np.float32).reshape(Q, H, D)
    k = np.asarray(key_states, dtype=np.float32).reshape(KV, D)
    v = np.asarray(value_states, dtype=np.float32).reshape(KV, D)

    kT = np.ascontiguousarray(k.T)  # [128, KV]
    # [V | ones] in fp16, laid out [128 kv-local, NKV * 129]
    va = np.concatenate(
        [v.reshape(NKV, P, D), np.ones((NKV, P, 1), np.float32)], axis=2
    ).astype(np.float16)
    vaug = np.ascontiguousarray(va.transpose(1, 0, 2)).reshape(P, NKV * VA)

    in_maps = []
    for c in range(N_CORES):
        # units: (head, q-half) -> qT [NU, 128, QU]
        qTc = np.empty((NU, P, QU), np.float32)
        for hh in range(HPC):
            qh = np.ascontiguousarray(q[:, c * HPC + hh, :].T)  # [128, Q]
            qTc[hh * 2] = qh[:, 0:QU]
            qTc[hh * 2 + 1] = qh[:, QU:]
        in_maps.append({"qT": qTc, "kT": kT, "vaug": vaug})

    nc = _get_nc()
    res = run_bass_kernel_spmd(nc, in_maps, core_ids=list(range(N_CORES)))

    out = np.empty((Q, H, D), dtype=np.float32)
    for c in range(N_CORES):
        oc = res.results[c]["o"]  # [NU, NQO, 128, D]
        for hh in range(HPC):
            o_head = np.concatenate([oc[hh * 2], oc[hh * 2 + 1]], axis=0)
            out[:, c * HPC + hh, :] = o_head.reshape(Q, D)
    return out.reshape(1, Q, H, D)


# revision 5
# speedup vs baseline: 2.0170x; 1.1112x over previous
"""MQA attention kernel for Trainium2, sharded over 8 NeuronCores.

Problem: query [1, 2048, 16, 128] f32, shared key/value [1, 2048, 128] f32,
mask [1, 16, 2048, 2048] bool (all ones -> no-op, per problem spec fill).

Sharding: tensor-parallel over heads, 2 heads per core; K/V replicated.

Per-core kernel, software-pipelined over 4 units (head x q-half, q=1024):
  - scores^T stripes: S^T[kv_tile, q_unit] = K^T(stationary) @ Q^T(moving),
    fp32r matmuls, contraction d=128 on partitions.
  - P^T = exp(SCALE * S^T) on ScalarE, fp32 PSUM -> fp16 SBUF.
  - PV: out[q, 0:128] = attention numerator, out[q, 128] = softmax denominator,
    in ONE accumulation group per q-chunk of 128: lhsT = P^T tile (stationary),
    rhs = [V | ones] (moving, fp16). No on-chip transposes anywhere.
  - normalize with DVE reciprocal + tensor_scalar_mul while evacuating PSUM.
Unit u's PV groups are interleaved (in program order) with unit u+1's
scores/exp so the PE stays dense while ScalarE (the exp floor) streams.

Host side: pre-transposes Q/K (free on CPU), appends the ones column to V,
scatters per-core inputs, gathers per-core outputs.
"""

import numpy as np

import concourse.bass as bass
import concourse.tile as tile
from concourse import bacc, mybir
from concourse.bass_utils import run_bass_kernel_spmd

N_CORES = 8
H = 16
HPC = H // N_CORES   # heads per core
Q = 2048
KV = 2048
D = 128
P = 128
QU = 1024            # q extent of one pipeline unit
NU = HPC * Q // QU   # 4 pipeline units per core
NKV = KV // P        # 16 kv tiles
NQS = QU // 512      # 2 scores matmuls (N=512) per psum stripe
NQO = QU // P        # 8 PV q-chunks per unit
VA = D + 1           # V augmented with a ones column
SCALE = float(1.0 / np.sqrt(np.float32(D)))

F32 = mybir.dt.float32
F32R = mybir.dt.float32r
F16 = mybir.dt.float16

_CACHE = {}


def _build():
    nc = bacc.Bacc("TRN2", target_bir_lowering=False, debug=False,
                   num_devices=N_CORES)
    qT = nc.dram_tensor("qT", [NU, P, QU], F16, kind="ExternalInput")
    kT = nc.dram_tensor("kT", [P, KV], F16, kind="ExternalInput")
    vaug = nc.dram_tensor("vaug", [P, NKV * VA], F16, kind="ExternalInput")
    o = nc.dram_tensor("o", [NU, NQO, P, D], F32, kind="ExternalOutput")

    with tile.TileContext(nc) as tc:
        with (
            tc.tile_pool(name="const", bufs=1) as const_pool,
            tc.tile_pool(name="qTp", bufs=2) as qT_pool,
            tc.tile_pool(name="pT", bufs=2 * NKV) as pT_pool,
            tc.tile_pool(name="osb", bufs=2) as osb_pool,
            tc.tile_pool(name="recip", bufs=4) as recip_pool,
            tc.tile_pool(name="psumS", bufs=3, space="PSUM") as psumS_pool,
            tc.tile_pool(name="psumO", bufs=2, space="PSUM") as psumO_pool,
        ):
            # split big input DMAs so the first matmul can start early
            kT_sb = const_pool.tile([P, KV], F16)
            nc.sync.dma_start(kT_sb[:], kT.ap())
            vaug_sb = const_pool.tile([P, NKV * VA], F16)

            qT_sbs = {}

            def load_q(u):
                t = qT_pool.tile([P, QU], F16, name="qT_sb", tag="qT")
                nc.sync.dma_start(t[:], qT.ap()[u])
                qT_sbs[u] = t

            load_q(0)
            nc.sync.dma_start(vaug_sb[:], vaug.ap())
            load_q(1)

            pTs = {u: [] for u in range(NU)}
            osbs = {}

            def pv_group(u, j):
                # one PSUM accumulation group: O[q_j, :] plus denominator
                po = psumO_pool.tile([P, VA], F32, name="po", tag="po")
                for i in range(NKV):
                    nc.tensor.matmul(
                        po[:],
                        pTs[u][i][:, j * P:(j + 1) * P],
                        vaug_sb[:, i * VA:(i + 1) * VA],
                        start=(i == 0), stop=(i == NKV - 1),
                    )
                rc = recip_pool.tile([P, 1], F32, name="rc", tag="rc")
                nc.vector.reciprocal(rc[:], po[:, D:D + 1])
                nc.vector.tensor_scalar_mul(
                    osbs[u][:, j * P:(j + 1) * P], po[:, 0:D], rc[:],
                )

            for u in range(NU + 1):
                if u < NU:
                    osbs[u] = osb_pool.tile([P, QU], F32, name="osb", tag="osb")
                for i in range(NKV):
                    # scores + exp for unit u
                    if u < NU:
                        ps = psumS_pool.tile([P, QU], F32, name="ps", tag="ps")
                        for j in range(NQS):
                            nc.tensor.matmul(
                                ps[:, j * 512:(j + 1) * 512],
                                kT_sb[:, i * P:(i + 1) * P],
                                qT_sbs[u][:, j * 512:(j + 1) * 512],
                                start=True, stop=True,
                            )
                        pT = pT_pool.tile([P, QU], F16, name="pT", tag="pT")
                        nc.scalar.activation(
                            pT[:], ps[:], mybir.ActivationFunctionType.Exp,
                            scale=SCALE,
                        )
                        pTs[u].append(pT)
                    # PV for unit u-1, one group every other kv-tile
                    if u > 0 and i % 2 == 0:
                        pv_group(u - 1, i // 2)
                        if i // 2 == NQO // 2 - 1 or i // 2 == NQO - 1:
                            half = 0 if i // 2 == NQO // 2 - 1 else 1
                            lo, hi = half * NQO // 2, (half + 1) * NQO // 2
                            nc.sync.dma_start(
                                o.ap()[u - 1][lo:hi].rearrange("j p d -> p j d"),
                                osbs[u - 1][:, lo * D:hi * D].rearrange(
                                    "p (j d) -> p j d", d=D),
                            )
                if u == 0:
                    load_q(2)
                if u == 1:
                    load_q(3)
                if u > 0:
                    pTs[u - 1] = []
    nc.compile()
    return nc


def _get_nc():
    if "nc" not in _CACHE:
        _CACHE["nc"] = _build()
    return _CACHE["nc"]


def kernel(query_states, key_states, value_states, attention_mask):
    # mask is all-ones by problem construction -> identity; ignored.
    q = np.asarray(query_states, dtype=np.float32).reshape(Q, H, D)
    k = np.asarray(key_states, dtype=np.float32).reshape(KV, D)
    v = np.asarray(value_states, dtype=np.float32).reshape(KV, D)

    kT = np.ascontiguousarray(k.T).astype(np.float16)  # [128, KV]
    # [V | ones] in fp16, laid out [128 kv-local, NKV * 129]
    va = np.concatenate(
        [v.reshape(NKV, P, D), np.ones((NKV, P, 1), np.float32)], axis=2
    ).astype(np.float16)
    vaug = np.ascontiguousarray(va.transpose(1, 0, 2)).reshape(P, NKV * VA)

    in_maps = []
    for c in range(N_CORES):
        # units: (head, q-half) -> qT [NU, 128, QU]
        qTc = np.empty((NU, P, QU), np.float16)
        for hh in range(HPC):
            qh = np.ascontiguousarray(q[:, c * HPC + hh, :].T)  # [128, Q]
            qTc[hh * 2] = qh[:, 0:QU]
            qTc[hh * 2 + 1] = qh[:, QU:]
        in_maps.append({"qT": qTc, "kT": kT, "vaug": vaug})

    nc = _get_nc()
    res = run_bass_kernel_spmd(nc, in_maps, core_ids=list(range(N_CORES)))

    out = np.empty((Q, H, D), dtype=np.float32)
    for c in range(N_CORES):
        oc = res.results[c]["o"]  # [NU, NQO, 128, D]
        for hh in range(HPC):
            o_head = np.concatenate([oc[hh * 2], oc[hh * 2 + 1]], axis=0)
            out[:, c * HPC + hh, :] = o_head.reshape(Q, D)
    return out.reshape(1, Q, H, D)
